# revision 96
# baseline (speedup 1.0000x reference)
# Multi-head causal attention (B=1, T=4096, D=1024, H=16) on 8 TRN2 NeuronCores.
#
# Sharding: tensor-parallel over heads. Core n computes head channels
# [128n, 128n+128) (= heads 2n, 2n+1), runs the full causal attention for its
# two heads, and produces a full-shape partial output
#   y_n = attn_out[:, ch_n] @ Wo[:, ch_n].T        (4096, 1024)
# The host sums the 8 partials (row-sharded Wo contraction) — no collectives.
#
# Device-side schedule: the PE must never idle (HAM clock-gate throttles
# 2.4->1.2 GHz after idle windows) and the ACT engine's exp is a hard floor
# (~1 elem/cycle/lane).  Key structure:
#  - (pair, head) micro-steps with per-head 2-bank score tiles: head h's next
#    scores run on the PE while the other head's exp occupies ACT, so the
#    exp->scores->exp chain never stalls ACT (chain-free phasing).
#  - K is stored zero-padded per head ([128, 2, T]: head h's 64 channels on
#    its own partitions, zeros elsewhere) so score matmuls are full-K=128 --
#    fast-weight-load + background LDWEIGHTS apply (216ns/mm instead of 317).
#  - QKV projections (next chunk), output projections and softmax tails are
#    "filler thunks" popped into the PE queue between attention steps; the
#    tail's no-deadline stages (normalize/y) go to a deferred queue drained
#    against a global slot budget, shifting PE work from the PE-starved early
#    (small-causal-wedge) chunks into the ACT-bound late chunks.
#  - PSUM: per-head score tiles 2x2 banks + 2 PV accumulator banks +
#    2 proj/tail banks = 8.  Filler thunks sharing the proj tag must be
#    popped in execution-readiness order (tag allocation order == WAR order).
#
# Softmax: the ones-column appended to V (M=65) makes psum row 64 the rowsum;
# normalization via a partition-spread SBUF->SBUF DMA + DVE reciprocal +
# broadcast DMA through DRAM scratch (all hidden in the pipeline); the final
# chunk's exposed tail instead uses an all-on-chip path (reciprocal_approx_
# fast + K=1 broadcast matmuls).
#
# Also: causally-dead query columns are trimmed from diagonal score/PV
# matmuls, and ~3.4us of dummy matmuls during the initial DMA wait flip the
# HAM clock-gate to 2.4 GHz before the first real projections.
#
# Measured on 8 axon TRN2 cores: ~220 us HW exec (vs 343 us baseline),
# rel L2 error vs fp32 reference ~6.1e-3.

import os
import sys

for _p in ("/opt/trn_rl_repo", "/root/.axon_site/_ro/trn_rl_repo"):
    if os.path.isdir(_p) and _p not in sys.path:
        sys.path.insert(0, _p)

import ml_dtypes
import numpy as np


def _ensure_axon_ntff_hook():
    """The agent image's antenv package lacks axon_hooks, which makes
    run_bass_kernel_spmd(trace=True) crash at import under axon. Provide the
    module and register the boot hook so NTFF profiling works."""
    import types

    try:
        import antenv.axon_hooks  # noqa: F401
        return
    except ImportError:
        pass
    try:
        import antenv
    except ImportError:
        return
    mod = types.ModuleType("antenv.axon_hooks")
    mod._hook = None
    mod.set_axon_ntff_profile_hook = lambda h: setattr(mod, "_hook", h)
    mod.get_axon_ntff_profile_hook = lambda: mod._hook
    sys.modules["antenv.axon_hooks"] = mod
    antenv.axon_hooks = mod
    try:
        from trn_agent_boot.trn_boot import _ntff_profile_via_ctypes

        so = "/opt/axon/libaxon_pjrt.so"
        if os.path.exists(so):
            mod._hook = _ntff_profile_via_ctypes(so)
    except Exception:
        pass


_ensure_axon_ntff_hook()

import concourse.bass as bass  # noqa: E402
import concourse.tile as tile  # noqa: E402
from concourse import bacc  # noqa: E402
from concourse import mybir  # noqa: E402
from concourse.bass_utils import run_bass_kernel_spmd  # noqa: E402

F32 = mybir.dt.float32
BF16 = mybir.dt.bfloat16
EXP = mybir.ActivationFunctionType.Exp
NPBF = ml_dtypes.bfloat16

D = 1024          # d_model
DK = 64           # head dim
CPC = 128         # channels per core (2 heads)
ICH = 512         # query-chunk size (= psum bank free width in fp32)
JT = 128          # key-tile size

_NC_CACHE = {}


def build(T):
    """Build the per-core Bass program for sequence length T."""
    nc = bacc.Bacc(None, target_bir_lowering=False, debug=False)
    nch = T // ICH
    KD = D // 128  # contraction tiles for the projections

    xT_d = nc.dram_tensor("xT", [D, T], BF16, kind="ExternalInput")
    wqT_d = nc.dram_tensor("wqT", [D, CPC], BF16, kind="ExternalInput")
    wkT_d = nc.dram_tensor("wkT", [D, CPC], BF16, kind="ExternalInput")
    wvT_d = nc.dram_tensor("wvT", [D, CPC], BF16, kind="ExternalInput")
    woT_d = nc.dram_tensor("woT", [CPC, D], BF16, kind="ExternalInput")
    tri_d = nc.dram_tensor("tri", [JT, JT], BF16, kind="ExternalInput")
    ident_d = nc.dram_tensor("ident", [128, 128], BF16, kind="ExternalInput")
    y_d = nc.dram_tensor("y", [T, D], BF16, kind="ExternalOutput")
    rs2_scratch = nc.dram_tensor("rs2_scratch", [nch, 2 * ICH], F32)

    with tile.TileContext(nc) as tc:
        with (
            tc.tile_pool(name="const", bufs=1) as const,
            tc.tile_pool(name="xtp", bufs=3) as xtp,
            tc.tile_pool(name="vtp", bufs=2) as vtp,
            tc.tile_pool(name="expp", bufs=4) as expp,
            tc.tile_pool(name="outp", bufs=8) as outp,
            tc.tile_pool(name="yp", bufs=4) as yp,
            tc.tile_pool(name="scp", bufs=1, space="PSUM") as scp,
            tc.tile_pool(name="prp", bufs=1, space="PSUM") as prp,
            tc.tile_pool(name="pvp", bufs=1, space="PSUM") as pvp,
        ):
            # PSUM budget: per-head score tiles [128,2,512] x2 = 4 banks,
            # proj/tail 2x1, PV accumulators = 2.  The per-head score tiles
            # phase-shift the two heads: head h's next scores run on the PE
            # while the other head's exp occupies ACT, so ACT never waits.
            # proj/tail tiles alternate between two 1-bank tags so stage N+1's
            # matmuls overlap stage N's copy-out.
            def ring_tile(shape, dtype, name):
                return prp.tile(shape, dtype, tag="proj", name=name)
            # ---- constants / persistent state ----
            wq_sb = const.tile([128, KD, 128], BF16)
            wk_sb = const.tile([128, KD, 128], BF16)
            wv_sb = const.tile([128, KD, 128], BF16)
            for w_sb, w_d in ((wq_sb, wqT_d), (wk_sb, wkT_d), (wv_sb, wvT_d)):
                nc.sync.dma_start(
                    out=w_sb, in_=w_d.rearrange("(t p) c -> p t c", p=128)
                )
            wo_sb = const.tile([128, D], BF16)
            nc.sync.dma_start(out=wo_sb, in_=woT_d[:, :])
            tri_sb = const.tile([JT, JT], BF16)
            nc.sync.dma_start(out=tri_sb, in_=tri_d[:, :])
            id_sb = const.tile([128, 128], BF16)
            nc.sync.dma_start(out=id_sb, in_=ident_d[:, :])
            ones33 = const.tile([33, DK], BF16)
            nc.vector.memset(ones33, 1.0)

            q_sb = const.tile([128, T], BF16)  # QT, both heads stacked
            # K in zero-padded per-head layout: [:, h, :] holds head h's K on
            # its own 64 partitions and zeros on the other 64, so score
            # matmuls are full-K=128 (FWL + background weight-load apply) and
            # both heads stream the same rhs region.  (Row-tiled K=64 pairs
            # were tried: the pair concurrency is real but the extra Q/K
            # layout copies at chunk starts cost 3x what it saved.)
            ktz_sb = const.tile([128, 2, T], BF16)
            nc.vector.memset(ktz_sb, 0.0)
            # V' = [V_h | 1] per head: [j, jt, 2*65]
            vp_sb = const.tile([128, T // JT, 2 * (DK + 1)], BF16)
            ones_view = vp_sb.rearrange("p t (h c) -> p t h c", h=2)[
                :, :, :, DK : DK + 1
            ]
            nc.vector.memset(ones_view, 1.0)

            xT_v = xT_d.rearrange("(t p) i -> p t i", p=128)
            xt_tiles = {}

            # ---------- projection thunks for chunk c ----------
            def proj_thunks(c):
                i0 = c * ICH
                box = {}

                def f_load():
                    xt = xtp.tile([128, KD, ICH], BF16, tag="xt", name="xt_ch")
                    # two DMAs so the first Q matmuls start at half-load
                    h4 = KD // 2
                    nc.sync.dma_start(
                        out=xt[:, 0:h4, :], in_=xT_v[:, 0:h4, i0 : i0 + ICH]
                    )
                    nc.sync.dma_start(
                        out=xt[:, h4:KD, :], in_=xT_v[:, h4:KD, i0 : i0 + ICH]
                    )
                    xt_tiles[c] = xt

                def mk_qk(qk, w_sb, lo, hi):
                    def f():
                        if qk == 0 and lo == 0:
                            box["qk"] = ring_tile([128, 2, ICH], F32, "qk_ps")
                        for t in range(lo, hi):
                            nc.tensor.matmul(
                                out=box["qk"][:, qk, :],
                                lhsT=w_sb[:, t, :],
                                rhs=xt_tiles[c][:, t, :],
                                start=(t == 0),
                                stop=(t == KD - 1),
                            )

                    return f

                def f_qk_copy():
                    nc.vector.tensor_copy(
                        out=q_sb[:, i0 : i0 + ICH], in_=box["qk"][:, 0, :]
                    )
                    for h in range(2):
                        hp = slice(h * DK, (h + 1) * DK)
                        nc.vector.tensor_copy(
                            out=ktz_sb[hp, h, i0 : i0 + ICH],
                            in_=box["qk"][hp, 1, :],
                        )

                def mk_v(lo, hi):
                    def f():
                        if lo == 0:
                            box["vt"] = ring_tile([128, ICH], F32, "vt_ps")
                        for t in range(lo, hi):
                            nc.tensor.matmul(
                                out=box["vt"],
                                lhsT=wv_sb[:, t, :],
                                rhs=xt_tiles[c][:, t, :],
                                start=(t == 0),
                                stop=(t == KD - 1),
                            )

                    return f

                def f_vt_copy():
                    vt_sb = vtp.tile([128, ICH], BF16, tag="vt", name="vt_sb")
                    box["vtsb"] = vt_sb
                    nc.vector.tensor_copy(out=vt_sb, in_=box["vt"])

                def f_transp():
                    vn_ps = ring_tile([128, ICH // 128, 128], BF16, "vn_ps")
                    for sdx in range(ICH // 128):
                        nc.tensor.transpose(
                            out=vn_ps[:, sdx, :],
                            in_=box["vtsb"][:, sdx * 128 : (sdx + 1) * 128],
                            identity=id_sb,
                        )
                    jt0 = i0 // JT
                    nc.vector.tensor_copy(
                        out=vp_sb.rearrange("p t (h c) -> p t h c", h=2)[
                            :, jt0 : jt0 + ICH // 128, :, 0:DK
                        ],
                        in_=vn_ps.rearrange("p s (h c) -> p s h c", h=2),
                    )
                    del xt_tiles[c]

                qt = KD // 4
                return (
                    [f_load]
                    + [mk_qk(0, wq_sb, i * qt, (i + 1) * qt) for i in range(4)]
                    + [mk_qk(1, wk_sb, i * qt, (i + 1) * qt) for i in range(4)]
                    + [f_qk_copy]
                    + [mk_v(i * qt, (i + 1) * qt) for i in range(4)]
                    + [f_vt_copy, f_transp]
                )

            # ---------- tail thunks for chunk c ----------
            def tail_thunks(c, pv, final=False):
                i0 = c * ICH
                box = {}

                def t_rs():
                    # rowsums (psum row 64 of each head) -> spread across 128
                    # partitions (SBUF->SBUF DMA) for a fast reciprocal
                    rs = outp.tile([DK + 1, 2, ICH], F32, tag="rs", name="rs")
                    box["rs"] = rs
                    for h in range(2):
                        nc.vector.tensor_copy(
                            out=rs[DK : DK + 1, h, :], in_=pv[h][DK : DK + 1, :]
                        )

                def t_rs_final():
                    # all-on-chip: rowsums to partitions 0/32, one wide recip,
                    # then K=1 broadcast matmuls (no DMA hops on the critical
                    # path of the exposed last tail)
                    rs = outp.tile([33, ICH], F32, tag="rs", name="rsf")
                    for h in range(2):
                        # ScalarE: ACT is done with exps by now; reads PSUM
                        # faster and overlaps with DVE
                        nc.scalar.copy(
                            out=rs[32 * h : 32 * h + 1, :],
                            in_=pv[h][DK : DK + 1, :],
                        )
                    nc.vector.reciprocal_approx_fast(out=rs, in_=rs)
                    rsb = outp.tile([33, ICH], BF16, tag="rsb", name="rsb")
                    nc.vector.tensor_copy(out=rsb, in_=rs)
                    bc_ps = ring_tile([128, ICH], F32, "bc_ps")
                    box["bc"] = bc_ps
                    for h in range(2):
                        nc.tensor.matmul(
                            out=bc_ps[h * DK : (h + 1) * DK, :],
                            lhsT=ones33[32 * h : 32 * h + 1, :],
                            rhs=rsb[32 * h : 32 * h + 1, :],
                            start=True,
                            stop=True,
                        )

                def t_outt():
                    outt = outp.tile([128, ICH], BF16, tag="outt", name="outt")
                    box["outt"] = outt
                    for h in range(2):
                        # final tail: ScalarE is idle and reads PSUM faster;
                        # frees DVE for the rowsum/recip chain
                        eng = nc.scalar if final else nc.vector
                        if final:
                            eng.copy(
                                out=outt[h * DK : (h + 1) * DK, :],
                                in_=pv[h][0:DK, :],
                            )
                        else:
                            nc.vector.tensor_copy(
                                out=outt[h * DK : (h + 1) * DK, :],
                                in_=pv[h][0:DK, :],
                            )

                def t_recip():
                    rsp = outp.tile(
                        [128, 2 * ICH // 128], F32, tag="rsp", name="rsp"
                    )
                    nc.sync.dma_start(
                        out=rsp, in_=box["rs"][DK : DK + 1, :, :]
                    )
                    nc.vector.reciprocal(out=rsp, in_=rsp)
                    nc.sync.dma_start(
                        out=rs2_scratch[c].rearrange("(p f) -> p f", p=128),
                        in_=rsp,
                    )

                def t_bc():
                    bc = outp.tile([128, ICH], F32, tag="bc", name="bc")
                    box["bc"] = bc
                    for h in range(2):
                        nc.sync.dma_start(
                            out=bc[h * DK : (h + 1) * DK, :],
                            in_=rs2_scratch[c]
                            .rearrange("(h i) -> h i", h=2)[h : h + 1, :]
                            .to_broadcast([DK, ICH]),
                        )

                def t_mul():
                    nc.vector.tensor_mul(box["outt"], box["outt"], box["bc"])

                def mk_y(sidx):
                    def t_y():
                        # the final chunk's y stages borrow the freed score
                        # banks so the 4 stages don't serialize on one tag
                        if final:
                            y_ps = scp.tile(
                                [128, 2, ICH], F32,
                                tag=f"sc{sidx % 2}", name="y_ps",
                            )
                        else:
                            y_ps = ring_tile([128, 2, ICH], F32, "y_ps")
                        for e in range(2):
                            nc.tensor.matmul(
                                out=y_ps[:, e, :],
                                lhsT=box["outt"][
                                    :, sidx * 128 : (sidx + 1) * 128
                                ],
                                rhs=wo_sb[:, e * ICH : (e + 1) * ICH],
                                start=True,
                                stop=True,
                            )
                        y_sb = yp.tile([128, D], BF16, tag="y", name="y_sb")
                        nc.vector.tensor_copy(
                            out=y_sb.rearrange("p (e i) -> p e i", e=2),
                            in_=y_ps,
                        )
                        r0 = i0 + sidx * 128
                        nc.sync.dma_start(out=y_d[r0 : r0 + 128, :], in_=y_sb)

                    return t_y

                if final:
                    return [t_rs_final, t_outt, t_mul] + [
                        mk_y(s) for s in range(ICH // 128)
                    ]
                return [t_rs, t_outt, t_recip, t_bc, t_mul] + [
                    mk_y(s) for s in range(ICH // 128)
                ]

            # ---------- main schedule ----------
            # filler entries: ('proj', f) must drain before the next chunk's
            # attention (emission-order deadlock otherwise); ('tail', f) may
            # spill into later chunks.  `deferred` holds tail stages with no
            # deadline (normalize/y-projection) — they only pop once filler is
            # empty, shifting PE work from the PE-starved early chunks into
            # the ACT-bound late chunks.
            filler = []
            deferred = []

            def pop_filler(n=1):
                for _ in range(n):
                    if filler:
                        filler.pop(0)[1]()
                    elif deferred:
                        deferred.pop(0)()

            def drain_proj():
                rest = []
                for kind, f in filler:
                    if kind == "proj":
                        f()
                    else:
                        rest.append((kind, f))
                filler[:] = rest

            def emit_pv_h(pv, h, p, ex, i0, njt):
                for jj in range(2):
                    jt = 2 * p + jj
                    off = max(0, jt * JT - i0)
                    nc.tensor.matmul(
                        out=pv[h][0 : DK + 1, off:],
                        lhsT=vp_sb[:, jt, h * (DK + 1) : (h + 1) * (DK + 1)],
                        rhs=ex[:, jj, off:],
                        start=(jt == 0),
                        stop=(jt == njt - 1),
                    )

            # PE warm-up: ~3.4us of dummy matmuls during the initial weight/x
            # DMA wait flips the HAM clock-gate to 2.4 GHz before the real
            # projection matmuls start (otherwise the first ~16 run at 1.2)
            # parked on a PV bank: its first real use (chunk 0's accumulator)
            # is ~12us in, so the warm-up never WAR-blocks the projections
            warm_ps = pvp.tile([128, ICH], F32, tag="pv0", name="warm_ps")
            for _ in range(28):
                nc.tensor.matmul(
                    out=warm_ps[:, 0:128],
                    lhsT=id_sb,
                    rhs=id_sb,
                    start=True,
                    stop=True,
                )

            # prologue: only load + Q/K of chunk 0 run serially; its V phase
            # interleaves with chunk 0's first score/exp steps
            _pj0 = proj_thunks(0)
            for f in _pj0[:10]:
                f()
            filler.extend([("proj", f) for f in _pj0[10:]])

            prev = None
            for c in range(nch):
                njt = (c + 1) * (ICH // JT)
                i0 = c * ICH
                pv = [
                    pvp.tile([128, ICH], F32, tag=f"pv{h}", name=f"pv{h}")
                    for h in range(2)
                ]
                tl = list(tail_thunks(*prev)) if prev is not None else []
                pj = [("proj", f) for f in (proj_thunks(c + 1) if c + 1 < nch else [])]
                # Order matters twice over: thunks sharing the proj PSUM tag
                # allocate in pop order (allocation order must match execution
                # readiness), and the tail's normalize/mul stages sit behind a
                # multi-hop DMA chain, so the proj Q/K phase goes in between
                # to hide that latency.  The x prefetch DMA goes first so its
                # latency hides under the tail instead of stalling the Q mms.
                # rs/outt (releasing the PV banks) stay urgent; the rest of
                # the tail has no deadline and goes to the deferred queue.
                filler.extend(pj[:1] + [("tail", f) for f in tl[:2]] + pj[1:])
                deferred.extend(tl[2:])

                nstep = njt  # (pair, head) micro-steps: njt//2 pairs x 2 heads
                # amortize deferred work over remaining chunks EXCLUDING the
                # last, so stragglers don't collide with the exposed final tail
                slots_all = max(
                    nstep,
                    sum((cc + 1) * (ICH // JT) for cc in range(c, nch - 1)),
                )
                prev_step = [None, None]  # per head: (p, ex)
                for s in range(nstep):
                    p, h = s // 2, s % 2
                    rem_slots = max(1, nstep - s)
                    # proj/urgent filler must drain within this chunk; the
                    # deferred tail work amortizes over all remaining chunks
                    per_slot = max(
                        -(-len(filler) // rem_slots),
                        -(-(len(filler) + len(deferred)) // max(1, slots_all - s)),
                    )
                    sc = scp.tile(
                        [128, 2, ICH], F32, tag=f"sc{h}", name=f"sc{h}"
                    )
                    for jj in range(2):
                        jt = 2 * p + jj
                        # queries below the diagonal offset are fully masked;
                        # don't stream them (the stale psum/ex region is never
                        # read: PV uses the same trimmed range)
                        off = max(0, jt * JT - i0)
                        nc.tensor.matmul(
                            out=sc[:, jj, off:],
                            lhsT=ktz_sb[:, h, jt * JT : (jt + 1) * JT],
                            rhs=q_sb[:, i0 + off : i0 + ICH],
                            start=True,
                            stop=True,
                        )
                    if prev_step[h] is not None:
                        emit_pv_h(pv, h, *prev_step[h], i0, njt)
                    pop_filler(per_slot)
                    ex = expp.tile(
                        [128, 2, ICH], BF16, tag=f"ex{h}", name=f"ex{h}"
                    )
                    offs = [max(0, (2 * p + jj) * JT - i0) for jj in range(2)]
                    if sum(offs) > 182:
                        # split exp to skip causally-dead columns (worth it
                        # only past the ~182-cycle per-instruction overhead)
                        for jj in range(2):
                            nc.scalar.activation(
                                out=ex[:, jj, offs[jj] :],
                                in_=sc[:, jj, offs[jj] :],
                                func=EXP,
                                scale=1.0 / np.sqrt(DK),
                            )
                    else:
                        nc.scalar.activation(
                            out=ex, in_=sc, func=EXP, scale=1.0 / np.sqrt(DK)
                        )
                    # causal mask on diagonal-straddling tiles
                    for jj in range(2):
                        jt = 2 * p + jj
                        off = jt * JT - i0
                        if off >= 0:
                            nc.vector.tensor_mul(
                                ex[:, jj, off : off + JT],
                                ex[:, jj, off : off + JT],
                                tri_sb,
                            )
                    prev_step[h] = (p, ex)
                for h in range(2):
                    emit_pv_h(pv, h, *prev_step[h], i0, njt)
                drain_proj()
                prev = (c, pv)
            while filler or deferred:
                pop_filler(1)
            for f in tail_thunks(*prev, final=True):
                f()

    nc.compile()
    return nc


def get_nc(T):
    if T not in _NC_CACHE:
        _NC_CACHE[T] = build(T)
    return _NC_CACHE[T]


TRI = np.triu(np.ones((JT, JT))).astype(NPBF)  # 1 where key j <= query i
IDENT = np.eye(128).astype(NPBF)

LAST_RESULTS = None  # BassKernelResults of the last run (for profiling)


def make_in_maps(x, Wq, Wk, Wv, Wo, n_cores=8):
    """x: (T, D) fp32. Returns per-core input maps (bf16 operands)."""
    xT = np.ascontiguousarray(x.T).astype(NPBF)
    maps = []
    for n in range(n_cores):
        sl = slice(CPC * n, CPC * (n + 1))
        maps.append(
            {
                "xT": xT,
                "wqT": np.ascontiguousarray(Wq[sl, :].T).astype(NPBF),
                "wkT": np.ascontiguousarray(Wk[sl, :].T).astype(NPBF),
                "wvT": np.ascontiguousarray(Wv[sl, :].T).astype(NPBF),
                "woT": np.ascontiguousarray(Wo[:, sl].T).astype(NPBF),
                "tri": TRI,
                "ident": IDENT,
            }
        )
    return maps


def run(x, Wq, Wk, Wv, Wo, T=None, n_cores=8, trace=False):
    global LAST_RESULTS
    T = T if T is not None else x.shape[0]
    nc = get_nc(T)
    in_maps = make_in_maps(x, Wq, Wk, Wv, Wo, n_cores)
    res = run_bass_kernel_spmd(
        nc, in_maps, core_ids=list(range(n_cores)), trace=trace
    )
    LAST_RESULTS = res
    y = np.zeros((T, D), dtype=np.float64)
    for r in res.results:
        y += r["y"].astype(np.float64)
    return y.astype(np.float32)


def kernel(x, Wq, Wk, Wv, Wo):
    x = np.asarray(x, dtype=np.float32)
    B, T, _ = x.shape
    trace = bool(os.environ.get("MHA_TRACE"))
    y = run(
        np.ascontiguousarray(x.reshape(T, D)),
        np.asarray(Wq, np.float32),
        np.asarray(Wk, np.float32),
        np.asarray(Wv, np.float32),
        np.asarray(Wo, np.float32),
        T=T,
        trace=trace,
    )
    if trace and LAST_RESULTS is not None and LAST_RESULTS.exec_time_ns:
        print(f"HW exec time: {LAST_RESULTS.exec_time_ns} ns")
    return y.reshape(B, T, D)


# revision 97
# speedup vs baseline: 1.0434x; 1.0434x over previous
# Multi-head causal attention (B=1, T=4096, D=1024, H=16) on 8 TRN2 NeuronCores.
#
# Sharding: tensor-parallel over heads. Core n computes head channels
# [128n, 128n+128) (= heads 2n, 2n+1), runs the full causal attention for its
# two heads, and produces a full-shape partial output
#   y_n = attn_out[:, ch_n] @ Wo[:, ch_n].T        (4096, 1024)
# The host sums the 8 partials (row-sharded Wo contraction) — no collectives.
#
# Device-side schedule: the PE must never idle (HAM clock-gate throttles
# 2.4->1.2 GHz after idle windows) and the ACT engine's exp is a hard floor
# (~1 elem/cycle/lane).  Key structure:
#  - (pair, head) micro-steps with per-head 2-bank score tiles: head h's next
#    scores run on the PE while the other head's exp occupies ACT, so the
#    exp->scores->exp chain never stalls ACT (chain-free phasing).
#  - K is stored zero-padded per head ([128, 2, T]: head h's 64 channels on
#    its own partitions, zeros elsewhere) so score matmuls are full-K=128 --
#    fast-weight-load + background LDWEIGHTS apply (216ns/mm instead of 317).
#  - QKV projections (next chunk), output projections and softmax tails are
#    "filler thunks" popped into the PE queue between attention steps; the
#    tail's no-deadline stages (normalize/y) go to a deferred queue drained
#    against a global slot budget, shifting PE work from the PE-starved early
#    (small-causal-wedge) chunks into the ACT-bound late chunks.
#  - PSUM: per-head score tiles 2x2 banks + 2 PV accumulator banks +
#    2 proj/tail banks = 8.  Filler thunks sharing the proj tag must be
#    popped in execution-readiness order (tag allocation order == WAR order).
#
# Softmax: the ones-column appended to V (M=65) makes psum row 64 the rowsum;
# normalization via a partition-spread SBUF->SBUF DMA + DVE reciprocal +
# broadcast DMA through DRAM scratch (all hidden in the pipeline); the final
# chunk's exposed tail instead uses an all-on-chip path (reciprocal_approx_
# fast + K=1 broadcast matmuls).
#
# Also: causally-dead query columns are trimmed from diagonal score/PV
# matmuls, and ~3.4us of dummy matmuls during the initial DMA wait flip the
# HAM clock-gate to 2.4 GHz before the first real projections.
#
# Measured on 8 axon TRN2 cores: ~220 us HW exec (vs 343 us baseline),
# rel L2 error vs fp32 reference ~6.1e-3.

import os
import sys

for _p in ("/opt/trn_rl_repo", "/root/.axon_site/_ro/trn_rl_repo"):
    if os.path.isdir(_p) and _p not in sys.path:
        sys.path.insert(0, _p)

import ml_dtypes
import numpy as np


def _ensure_axon_ntff_hook():
    """The agent image's antenv package lacks axon_hooks, which makes
    run_bass_kernel_spmd(trace=True) crash at import under axon. Provide the
    module and register the boot hook so NTFF profiling works."""
    import types

    try:
        import antenv.axon_hooks  # noqa: F401
        return
    except ImportError:
        pass
    try:
        import antenv
    except ImportError:
        return
    mod = types.ModuleType("antenv.axon_hooks")
    mod._hook = None
    mod.set_axon_ntff_profile_hook = lambda h: setattr(mod, "_hook", h)
    mod.get_axon_ntff_profile_hook = lambda: mod._hook
    sys.modules["antenv.axon_hooks"] = mod
    antenv.axon_hooks = mod
    try:
        from trn_agent_boot.trn_boot import _ntff_profile_via_ctypes

        so = "/opt/axon/libaxon_pjrt.so"
        if os.path.exists(so):
            mod._hook = _ntff_profile_via_ctypes(so)
    except Exception:
        pass


_ensure_axon_ntff_hook()

import concourse.bass as bass  # noqa: E402
import concourse.tile as tile  # noqa: E402
from concourse import bacc  # noqa: E402
from concourse import mybir  # noqa: E402
from concourse.bass_utils import run_bass_kernel_spmd  # noqa: E402

F32 = mybir.dt.float32
BF16 = mybir.dt.bfloat16
EXP = mybir.ActivationFunctionType.Exp
NPBF = ml_dtypes.bfloat16

D = 1024          # d_model
DK = 64           # head dim
CPC = 128         # channels per core (2 heads)
ICH = 512         # query-chunk size (= psum bank free width in fp32)
JT = 128          # key-tile size

_NC_CACHE = {}


def build(T):
    """Build the per-core Bass program for sequence length T."""
    nc = bacc.Bacc(None, target_bir_lowering=False, debug=False)
    nch = T // ICH
    KD = D // 128  # contraction tiles for the projections

    xT_d = nc.dram_tensor("xT", [D, T], BF16, kind="ExternalInput")
    wqT_d = nc.dram_tensor("wqT", [D, CPC], BF16, kind="ExternalInput")
    wkT_d = nc.dram_tensor("wkT", [D, CPC], BF16, kind="ExternalInput")
    wvT_d = nc.dram_tensor("wvT", [D, CPC], BF16, kind="ExternalInput")
    woT_d = nc.dram_tensor("woT", [CPC, D], BF16, kind="ExternalInput")
    tri_d = nc.dram_tensor("tri", [JT, JT], BF16, kind="ExternalInput")
    ident_d = nc.dram_tensor("ident", [128, 128], BF16, kind="ExternalInput")
    y_d = nc.dram_tensor("y", [T, D], BF16, kind="ExternalOutput")
    rs2_scratch = nc.dram_tensor("rs2_scratch", [nch, 2 * ICH], F32)

    with tile.TileContext(nc) as tc:
        with (
            tc.tile_pool(name="const", bufs=1) as const,
            tc.tile_pool(name="xtp", bufs=3) as xtp,
            tc.tile_pool(name="vtp", bufs=2) as vtp,
            tc.tile_pool(name="expp", bufs=4) as expp,
            tc.tile_pool(name="outp", bufs=8) as outp,
            tc.tile_pool(name="yp", bufs=4) as yp,
            tc.tile_pool(name="scp", bufs=1, space="PSUM") as scp,
            tc.tile_pool(name="prp", bufs=1, space="PSUM") as prp,
            tc.tile_pool(name="pvp", bufs=1, space="PSUM") as pvp,
        ):
            # PSUM budget: per-head score tiles [128,2,512] x2 = 4 banks,
            # proj/tail 2x1, PV accumulators = 2.  The per-head score tiles
            # phase-shift the two heads: head h's next scores run on the PE
            # while the other head's exp occupies ACT, so ACT never waits.
            # proj/tail tiles alternate between two 1-bank tags so stage N+1's
            # matmuls overlap stage N's copy-out.
            def ring_tile(shape, dtype, name):
                return prp.tile(shape, dtype, tag="proj", name=name)
            # ---- constants / persistent state ----
            wq_sb = const.tile([128, KD, 128], BF16)
            wk_sb = const.tile([128, KD, 128], BF16)
            wv_sb = const.tile([128, KD, 128], BF16)
            for w_sb, w_d in ((wq_sb, wqT_d), (wk_sb, wkT_d), (wv_sb, wvT_d)):
                nc.sync.dma_start(
                    out=w_sb, in_=w_d.rearrange("(t p) c -> p t c", p=128)
                )
            wo_sb = const.tile([128, D], BF16)
            nc.sync.dma_start(out=wo_sb, in_=woT_d[:, :])
            tri_sb = const.tile([JT, JT], BF16)
            nc.sync.dma_start(out=tri_sb, in_=tri_d[:, :])
            id_sb = const.tile([128, 128], BF16)
            nc.sync.dma_start(out=id_sb, in_=ident_d[:, :])
            ones33 = const.tile([33, DK], BF16)
            nc.vector.memset(ones33, 1.0)

            q_sb = const.tile([128, T], BF16)  # QT, both heads stacked
            # K in zero-padded per-head layout: [:, h, :] holds head h's K on
            # its own 64 partitions and zeros on the other 64, so score
            # matmuls are full-K=128 (FWL + background weight-load apply) and
            # both heads stream the same rhs region.  (Row-tiled K=64 pairs
            # were tried: the pair concurrency is real but the extra Q/K
            # layout copies at chunk starts cost 3x what it saved.)
            ktz_sb = const.tile([128, 2, T], BF16)
            nc.vector.memset(ktz_sb, 0.0)
            # V' = [V_h | 1] per head: [j, jt, 2*65]
            vp_sb = const.tile([128, T // JT, 2 * (DK + 1)], BF16)
            ones_view = vp_sb.rearrange("p t (h c) -> p t h c", h=2)[
                :, :, :, DK : DK + 1
            ]
            nc.vector.memset(ones_view, 1.0)

            xT_v = xT_d.rearrange("(t p) i -> p t i", p=128)
            xt_tiles = {}

            # ---------- projection thunks for chunk c ----------
            def proj_thunks(c):
                i0 = c * ICH
                box = {}

                def f_load():
                    xt = xtp.tile([128, KD, ICH], BF16, tag="xt", name="xt_ch")
                    # two DMAs so the first Q matmuls start at half-load
                    h4 = KD // 2
                    nc.sync.dma_start(
                        out=xt[:, 0:h4, :], in_=xT_v[:, 0:h4, i0 : i0 + ICH]
                    )
                    nc.sync.dma_start(
                        out=xt[:, h4:KD, :], in_=xT_v[:, h4:KD, i0 : i0 + ICH]
                    )
                    xt_tiles[c] = xt

                def mk_qk(qk, w_sb, lo, hi):
                    def f():
                        if qk == 0 and lo == 0:
                            box["qk"] = ring_tile([128, 2, ICH], F32, "qk_ps")
                        for t in range(lo, hi):
                            nc.tensor.matmul(
                                out=box["qk"][:, qk, :],
                                lhsT=w_sb[:, t, :],
                                rhs=xt_tiles[c][:, t, :],
                                start=(t == 0),
                                stop=(t == KD - 1),
                            )

                    return f

                def f_qk_copy():
                    nc.vector.tensor_copy(
                        out=q_sb[:, i0 : i0 + ICH], in_=box["qk"][:, 0, :]
                    )
                    for h in range(2):
                        hp = slice(h * DK, (h + 1) * DK)
                        nc.vector.tensor_copy(
                            out=ktz_sb[hp, h, i0 : i0 + ICH],
                            in_=box["qk"][hp, 1, :],
                        )

                def mk_v(lo, hi):
                    def f():
                        if lo == 0:
                            box["vt"] = ring_tile([128, ICH], F32, "vt_ps")
                        for t in range(lo, hi):
                            nc.tensor.matmul(
                                out=box["vt"],
                                lhsT=wv_sb[:, t, :],
                                rhs=xt_tiles[c][:, t, :],
                                start=(t == 0),
                                stop=(t == KD - 1),
                            )

                    return f

                def f_vt_copy():
                    vt_sb = vtp.tile([128, ICH], BF16, tag="vt", name="vt_sb")
                    box["vtsb"] = vt_sb
                    nc.vector.tensor_copy(out=vt_sb, in_=box["vt"])

                def f_transp():
                    vn_ps = ring_tile([128, ICH // 128, 128], BF16, "vn_ps")
                    for sdx in range(ICH // 128):
                        nc.tensor.transpose(
                            out=vn_ps[:, sdx, :],
                            in_=box["vtsb"][:, sdx * 128 : (sdx + 1) * 128],
                            identity=id_sb,
                        )
                    jt0 = i0 // JT
                    nc.vector.tensor_copy(
                        out=vp_sb.rearrange("p t (h c) -> p t h c", h=2)[
                            :, jt0 : jt0 + ICH // 128, :, 0:DK
                        ],
                        in_=vn_ps.rearrange("p s (h c) -> p s h c", h=2),
                    )
                    del xt_tiles[c]

                qt = KD // 4
                return (
                    [f_load]
                    + [mk_qk(0, wq_sb, i * qt, (i + 1) * qt) for i in range(4)]
                    + [mk_qk(1, wk_sb, i * qt, (i + 1) * qt) for i in range(4)]
                    + [f_qk_copy]
                    + [mk_v(i * qt, (i + 1) * qt) for i in range(4)]
                    + [f_vt_copy, f_transp]
                )

            # ---------- tail thunks for chunk c ----------
            def tail_thunks(c, pv, final=False):
                i0 = c * ICH
                box = {}

                def t_rs():
                    # rowsums (psum row 64 of each head) -> spread across 128
                    # partitions (SBUF->SBUF DMA) for a fast reciprocal
                    rs = outp.tile([DK + 1, 2, ICH], F32, tag="rs", name="rs")
                    box["rs"] = rs
                    for h in range(2):
                        nc.vector.tensor_copy(
                            out=rs[DK : DK + 1, h, :], in_=pv[h][DK : DK + 1, :]
                        )

                def t_rs_final():
                    # all-on-chip: rowsums to partitions 0/32, one wide recip,
                    # then K=1 broadcast matmuls (no DMA hops on the critical
                    # path of the exposed last tail)
                    rs = outp.tile([33, ICH], F32, tag="rs", name="rsf")
                    for h in range(2):
                        # ScalarE: ACT is done with exps by now; reads PSUM
                        # faster and overlaps with DVE
                        nc.scalar.copy(
                            out=rs[32 * h : 32 * h + 1, :],
                            in_=pv[h][DK : DK + 1, :],
                        )
                    nc.vector.reciprocal_approx_fast(out=rs, in_=rs)
                    rsb = outp.tile([33, ICH], BF16, tag="rsb", name="rsb")
                    nc.vector.tensor_copy(out=rsb, in_=rs)
                    bc_ps = ring_tile([128, ICH], F32, "bc_ps")
                    box["bc"] = bc_ps
                    for h in range(2):
                        nc.tensor.matmul(
                            out=bc_ps[h * DK : (h + 1) * DK, :],
                            lhsT=ones33[32 * h : 32 * h + 1, :],
                            rhs=rsb[32 * h : 32 * h + 1, :],
                            start=True,
                            stop=True,
                        )

                def t_outt():
                    outt = outp.tile([128, ICH], BF16, tag="outt", name="outt")
                    box["outt"] = outt
                    for h in range(2):
                        # final tail: ScalarE is idle and reads PSUM faster;
                        # frees DVE for the rowsum/recip chain
                        eng = nc.scalar if final else nc.vector
                        if final:
                            eng.copy(
                                out=outt[h * DK : (h + 1) * DK, :],
                                in_=pv[h][0:DK, :],
                            )
                        else:
                            nc.vector.tensor_copy(
                                out=outt[h * DK : (h + 1) * DK, :],
                                in_=pv[h][0:DK, :],
                            )

                def t_recip():
                    rsp = outp.tile(
                        [128, 2 * ICH // 128], F32, tag="rsp", name="rsp"
                    )
                    nc.sync.dma_start(
                        out=rsp, in_=box["rs"][DK : DK + 1, :, :]
                    )
                    nc.vector.reciprocal(out=rsp, in_=rsp)
                    nc.sync.dma_start(
                        out=rs2_scratch[c].rearrange("(p f) -> p f", p=128),
                        in_=rsp,
                    )

                def t_bc():
                    bc = outp.tile([128, ICH], F32, tag="bc", name="bc")
                    box["bc"] = bc
                    for h in range(2):
                        nc.sync.dma_start(
                            out=bc[h * DK : (h + 1) * DK, :],
                            in_=rs2_scratch[c]
                            .rearrange("(h i) -> h i", h=2)[h : h + 1, :]
                            .to_broadcast([DK, ICH]),
                        )

                def t_mul():
                    nc.vector.tensor_mul(box["outt"], box["outt"], box["bc"])

                def mk_y(sidx):
                    def t_y():
                        # the final chunk's y stages borrow the freed score
                        # banks so the 4 stages don't serialize on one tag
                        if final:
                            y_ps = scp.tile(
                                [128, 2, ICH], F32,
                                tag=f"sc{sidx % 2}", name="y_ps",
                            )
                        else:
                            y_ps = ring_tile([128, 2, ICH], F32, "y_ps")
                        for e in range(2):
                            nc.tensor.matmul(
                                out=y_ps[:, e, :],
                                lhsT=box["outt"][
                                    :, sidx * 128 : (sidx + 1) * 128
                                ],
                                rhs=wo_sb[:, e * ICH : (e + 1) * ICH],
                                start=True,
                                stop=True,
                            )
                        y_sb = yp.tile([128, D], BF16, tag="y", name="y_sb")
                        nc.vector.tensor_copy(
                            out=y_sb.rearrange("p (e i) -> p e i", e=2),
                            in_=y_ps,
                        )
                        r0 = i0 + sidx * 128
                        nc.sync.dma_start(out=y_d[r0 : r0 + 128, :], in_=y_sb)

                    return t_y

                if final:
                    return [t_rs_final, t_outt, t_mul] + [
                        mk_y(s) for s in range(ICH // 128)
                    ]
                return [t_rs, t_outt, t_recip, t_bc, t_mul] + [
                    mk_y(s) for s in range(ICH // 128)
                ]

            # ---------- main schedule ----------
            # filler entries: ('proj', f) must drain before the next chunk's
            # attention (emission-order deadlock otherwise); ('tail', f) may
            # spill into later chunks.  `deferred` holds tail stages with no
            # deadline (normalize/y-projection) — they only pop once filler is
            # empty, shifting PE work from the PE-starved early chunks into
            # the ACT-bound late chunks.
            filler = []
            deferred = []

            def pop_filler(n=1):
                for _ in range(n):
                    if filler:
                        filler.pop(0)[1]()
                    elif deferred:
                        deferred.pop(0)()

            def drain_proj():
                rest = []
                for kind, f in filler:
                    if kind == "proj":
                        f()
                    else:
                        rest.append((kind, f))
                filler[:] = rest

            def emit_pv_h(pv, h, p, ex, i0, njt):
                for jj in range(2):
                    jt = 2 * p + jj
                    off = max(0, jt * JT - i0)
                    nc.tensor.matmul(
                        out=pv[h][0 : DK + 1, off:],
                        lhsT=vp_sb[:, jt, h * (DK + 1) : (h + 1) * (DK + 1)],
                        rhs=ex[:, jj, off:],
                        start=(jt == 0),
                        stop=(jt == njt - 1),
                    )

            # PE warm-up: ~3.4us of dummy matmuls during the initial weight/x
            # DMA wait flips the HAM clock-gate to 2.4 GHz before the real
            # projection matmuls start (otherwise the first ~16 run at 1.2)
            # parked on a PV bank: its first real use (chunk 0's accumulator)
            # is ~12us in, so the warm-up never WAR-blocks the projections
            warm_ps = pvp.tile([128, ICH], F32, tag="pv0", name="warm_ps")
            for _ in range(28):
                nc.tensor.matmul(
                    out=warm_ps[:, 0:128],
                    lhsT=id_sb,
                    rhs=id_sb,
                    start=True,
                    stop=True,
                )

            # prologue: only load + Q/K of chunk 0 run serially; its V phase
            # interleaves with chunk 0's first score/exp steps
            _pj0 = proj_thunks(0)
            for f in _pj0[:10]:
                f()
            filler.extend([("proj", f) for f in _pj0[10:]])

            prev = None
            for c in range(nch):
                njt = (c + 1) * (ICH // JT)
                i0 = c * ICH
                pv = [
                    pvp.tile([128, ICH], F32, tag=f"pv{h}", name=f"pv{h}")
                    for h in range(2)
                ]
                tl = list(tail_thunks(*prev)) if prev is not None else []
                pj = [("proj", f) for f in (proj_thunks(c + 1) if c + 1 < nch else [])]
                # Order matters twice over: thunks sharing the proj PSUM tag
                # allocate in pop order (allocation order must match execution
                # readiness), and the tail's normalize/mul stages sit behind a
                # multi-hop DMA chain, so the proj Q/K phase goes in between
                # to hide that latency.  The x prefetch DMA goes first so its
                # latency hides under the tail instead of stalling the Q mms.
                # rs/outt (releasing the PV banks) stay urgent; the rest of
                # the tail has no deadline and goes to the deferred queue.
                filler.extend(pj[:1] + [("tail", f) for f in tl[:2]] + pj[1:])
                deferred.extend(tl[2:])

                nstep = njt  # (pair, head) micro-steps: njt//2 pairs x 2 heads
                # amortize deferred work over remaining chunks EXCLUDING the
                # last, so stragglers don't collide with the exposed final tail
                slots_all = max(
                    nstep,
                    sum((cc + 1) * (ICH // JT) for cc in range(c, nch - 1)),
                )
                prev_step = [None, None]  # per head: (p, ex)
                for s in range(nstep):
                    p, h = s // 2, s % 2
                    rem_slots = max(1, nstep - s)
                    # proj/urgent filler must drain within this chunk; the
                    # deferred tail work amortizes over all remaining chunks
                    per_slot = max(
                        -(-len(filler) // rem_slots),
                        -(-(len(filler) + len(deferred)) // max(1, slots_all - s)),
                    )
                    sc = scp.tile(
                        [128, 2, ICH], F32, tag=f"sc{h}", name=f"sc{h}"
                    )
                    for jj in range(2):
                        jt = 2 * p + jj
                        # queries below the diagonal offset are fully masked;
                        # don't stream them (the stale psum/ex region is never
                        # read: PV uses the same trimmed range)
                        off = max(0, jt * JT - i0)
                        nc.tensor.matmul(
                            out=sc[:, jj, off:],
                            lhsT=ktz_sb[:, h, jt * JT : (jt + 1) * JT],
                            rhs=q_sb[:, i0 + off : i0 + ICH],
                            start=True,
                            stop=True,
                        )
                    if prev_step[h] is not None:
                        emit_pv_h(pv, h, *prev_step[h], i0, njt)
                    pop_filler(per_slot)
                    ex = expp.tile(
                        [128, 2, ICH], BF16, tag=f"ex{h}", name=f"ex{h}"
                    )
                    nc.scalar.activation(
                        out=ex, in_=sc, func=EXP, scale=1.0 / np.sqrt(DK)
                    )
                    # causal mask on diagonal-straddling tiles
                    for jj in range(2):
                        jt = 2 * p + jj
                        off = jt * JT - i0
                        if off >= 0:
                            nc.vector.tensor_mul(
                                ex[:, jj, off : off + JT],
                                ex[:, jj, off : off + JT],
                                tri_sb,
                            )
                    prev_step[h] = (p, ex)
                for h in range(2):
                    emit_pv_h(pv, h, *prev_step[h], i0, njt)
                drain_proj()
                prev = (c, pv)
            while filler or deferred:
                pop_filler(1)
            for f in tail_thunks(*prev, final=True):
                f()

    nc.compile()
    return nc


def get_nc(T):
    if T not in _NC_CACHE:
        _NC_CACHE[T] = build(T)
    return _NC_CACHE[T]


TRI = np.triu(np.ones((JT, JT))).astype(NPBF)  # 1 where key j <= query i
IDENT = np.eye(128).astype(NPBF)

LAST_RESULTS = None  # BassKernelResults of the last run (for profiling)


def make_in_maps(x, Wq, Wk, Wv, Wo, n_cores=8):
    """x: (T, D) fp32. Returns per-core input maps (bf16 operands)."""
    xT = np.ascontiguousarray(x.T).astype(NPBF)
    maps = []
    for n in range(n_cores):
        sl = slice(CPC * n, CPC * (n + 1))
        maps.append(
            {
                "xT": xT,
                "wqT": np.ascontiguousarray(Wq[sl, :].T).astype(NPBF),
                "wkT": np.ascontiguousarray(Wk[sl, :].T).astype(NPBF),
                "wvT": np.ascontiguousarray(Wv[sl, :].T).astype(NPBF),
                "woT": np.ascontiguousarray(Wo[:, sl].T).astype(NPBF),
                "tri": TRI,
                "ident": IDENT,
            }
        )
    return maps


def run(x, Wq, Wk, Wv, Wo, T=None, n_cores=8, trace=False):
    global LAST_RESULTS
    T = T if T is not None else x.shape[0]
    nc = get_nc(T)
    in_maps = make_in_maps(x, Wq, Wk, Wv, Wo, n_cores)
    res = run_bass_kernel_spmd(
        nc, in_maps, core_ids=list(range(n_cores)), trace=trace
    )
    LAST_RESULTS = res
    y = np.zeros((T, D), dtype=np.float64)
    for r in res.results:
        y += r["y"].astype(np.float64)
    return y.astype(np.float32)


def kernel(x, Wq, Wk, Wv, Wo):
    x = np.asarray(x, dtype=np.float32)
    B, T, _ = x.shape
    trace = bool(os.environ.get("MHA_TRACE"))
    y = run(
        np.ascontiguousarray(x.reshape(T, D)),
        np.asarray(Wq, np.float32),
        np.asarray(Wk, np.float32),
        np.asarray(Wv, np.float32),
        np.asarray(Wo, np.float32),
        T=T,
        trace=trace,
    )
    if trace and LAST_RESULTS is not None and LAST_RESULTS.exec_time_ns:
        print(f"HW exec time: {LAST_RESULTS.exec_time_ns} ns")
    return y.reshape(B, T, D)
